# revision 22
# baseline (speedup 1.0000x reference)
"""Trainium2 Bass kernel for nn_BasicBlock (3-layer GCN block with residual).

Math (per batch item b, per conv):
    out = A @ (x @ W) + bias,  A = normalized adjacency (with self loops)
where A[c, r] = sum over edges r->c of dinv[r]*dinv[c] (dense N x N, shared
across batch and precomputed on host from the edge lists).

Block:
    a1 = relu(A_sp @ (x  @ W1) + b1)
    a2 = relu(A_tm @ (a1 @ W2) + b2)
    o3 =      A_sp @ (a2 @ W3) + b3
    out = relu(o3 + x)

On-chip layouts per item (P=128 partitions):
    natural  [n, c] : node chunks on partitions           (rhs of A-matmul /
                                                           lhsT of form-iv)
    transposed [c, n]: channel chunks on partitions        (consumed by W-matmul)

Phases per item (matmul forms; AT = A^T so AT[m, n] = A[n, m]):
    1. g1T[c,n]  = sum_m x[m,c]  * AT_sp[m,n]      (lhsT=x chunk,  rhs=AT_sp)
    2. a1T[co,n] = relu(sum_ci W1[ci,co]*g1T[ci,n] + b1)   (lhsT=W1, rhs=g1T)
    3. h2[n,c]   = sum_ci a1T[ci,n] * W2[ci,c]     (lhsT=a1T chunk, rhs=W2)
    4. a2T[c,n]  = relu(sum_m h2[m,c]*AT_tm[m,n] + b2)
    5. h3[n,c]   = sum_ci a2T[ci,n] * W3[ci,c];  h3[N,:] = b3
    6. out[n,c]  = relu(sum_m AT_sp[m,n]*h3[m,c] + x[n,c])
       (AT_sp row N is all-ones over valid cols -> adds b3 to every node;
        harmless in phase 1 because x row N is zero-padded)

All matmuls bf16 (1 cycle/row on PE) with fp32 PSUM accumulation; residual is
added in fp32 from a fresh x DMA. Batch (64) is sharded 8 items/core over the
8 cores; A matrices / weights are replicated.
"""

import sys

if "/opt/trn_rl_repo" not in sys.path:
    sys.path.insert(0, "/opt/trn_rl_repo")

import numpy as np
import ml_dtypes

import concourse.bass as bass
import concourse.bacc as bacc
import concourse.mybir as mybir
import concourse.tile as tile
from concourse.bass_utils import run_bass_kernel_spmd

P = 128
B, N, C = 64, 1700, 256
N_CORES = 8
B_LOCAL = B // N_CORES

F32 = mybir.dt.float32
BF16 = mybir.dt.bfloat16
RELU = mybir.ActivationFunctionType.Relu
NP_BF16 = ml_dtypes.bfloat16


def _quarters(total, step=512):
    return [(q, min(step, total - q)) for q in range(0, total, step)]


def build_program(bl, n, c):
    """Build the Bass/Tile program for `bl` batch items, `n` nodes, `c` chans."""
    kt = -(-(n + 1) // P)  # node chunks; >= one pad row (bias row at index n)
    npad = kt * P
    ct = c // P
    nq = _quarters(npad)

    nqv = _quarters(n)  # valid-column quarters (phases whose pads are unread)

    nc = bacc.Bacc("TRN2", target_bir_lowering=False, debug=False,
                   enable_asserts=False)

    x_d = nc.dram_tensor("x", [bl, n, c], F32, kind="ExternalInput")
    atsp_d = nc.dram_tensor("at_sp", [P, kt, n], BF16, kind="ExternalInput")
    attm_d = nc.dram_tensor("at_tm", [P, kt, n], BF16, kind="ExternalInput")
    w_d = [nc.dram_tensor(f"w{i}", [P, ct, c], BF16, kind="ExternalInput")
           for i in (1, 2, 3)]
    b1_d = nc.dram_tensor("b1", [P, ct], F32, kind="ExternalInput")
    b2_d = nc.dram_tensor("b2", [P, ct], F32, kind="ExternalInput")
    b3_d = nc.dram_tensor("b3", [1, c], BF16, kind="ExternalInput")
    out_d = nc.dram_tensor("out", [bl, n, c], F32, kind="ExternalOutput")

    with tile.TileContext(nc) as tc:
        with (
            tc.tile_pool(name="const", bufs=1) as cpool,
            tc.tile_pool(name="xin", bufs=4) as xin,
            tc.tile_pool(name="xbf", bufs=3) as xbfp,
            tc.tile_pool(name="act", bufs=4) as actp,
            tc.tile_pool(name="h", bufs=3) as hp,
            tc.tile_pool(name="xres", bufs=4) as xresp,
            tc.tile_pool(name="outp", bufs=4) as outp,
            tc.tile_pool(name="psA", bufs=4, space="PSUM") as psA,
            tc.tile_pool(name="psW", bufs=4, space="PSUM") as psW,
        ):
            # --- constants.  Ring plan: at_sp is needed first (item-0
            # phase 1 consumes tile k at ~1.4*k us), so it is split across
            # the sync+vector HWDGE rings with at_tm queued behind it on the
            # same rings; x goes on the scalar ring; xr prefetches on
            # gpsimd; out stores on scalar. ---
            w_sb = []
            for i, wd in enumerate(w_d):
                w = cpool.tile([P, ct, c], BF16, tag=f"w{i}")
                nc.scalar.dma_start(w[:], wd[:])
                w_sb.append(w)
            b1_sb = cpool.tile([P, ct], F32, tag="b1")
            b2_sb = cpool.tile([P, ct], F32, tag="b2")
            nc.scalar.dma_start(b1_sb[:], b1_d[:])
            nc.scalar.dma_start(b2_sb[:], b2_d[:])

            at_sp = cpool.tile([P, kt, n], BF16, tag="at_sp")
            at_tm = cpool.tile([P, kt, n], BF16, tag="at_tm")
            for k in range(kt):
                eng = nc.sync if k % 2 == 0 else nc.scalar
                eng.dma_start(at_sp[:, k, :], atsp_d[:, k, :])
            for k in range(kt):
                eng = nc.sync if k % 2 == 0 else nc.scalar
                eng.dma_start(at_tm[:, k, :], attm_d[:, k, :])

            bias_tile = n // P      # global node index n == first pad row
            bias_part = n % P

            def emit_load_x(b):
                # load x, cast to bf16 (pad rows zero)
                x_eng = nc.gpsimd if b == 0 else nc.sync
                xbf = xbfp.tile([P, kt, c], BF16, tag="xbf", name=f"xbf_{b}")
                for k in range(kt):
                    rows = min(P, n - k * P)
                    if rows <= 0:
                        nc.vector.memset(xbf[:, k, :], 0)
                        continue
                    xf = xin.tile([P, c], F32, tag="xf", name=f"xf_{b}_{k}")
                    x_eng.dma_start(xf[:rows, :], x_d[b, k * P:k * P + rows, :])
                    if rows < P:
                        nc.vector.memset(xbf[:, k, :], 0)
                    nc.vector.tensor_copy(xbf[:rows, k, :], xf[:rows, :])
                return xbf

            def emit_p1(b, xbf):
                # phase 1: g1T = (A_sp @ x)^T
                g1T = actp.tile([P, ct, npad], BF16, tag="act", name=f"g1T_{b}")
                if b == 0:
                    # k-outer over 8 parallel PSUM banks so tile k of at_sp
                    # is consumed as soon as its DMA lands
                    groups = []
                    for cc in range(ct):
                        for qi, (q0, qs) in enumerate(nqv):
                            pool, tg = ((psA, "psA")
                                        if (cc * len(nqv) + qi) % 2 == 0
                                        else (psW, "psW"))
                            groups.append(
                                (pool.tile([P, 512], F32, tag=tg,
                                           name=f"ps1_{cc}_{qi}"), cc, q0, qs))
                    for k in range(kt):
                        for (ps, cc, q0, qs) in groups:
                            nc.tensor.matmul(
                                ps[:, :qs],
                                lhsT=xbf[:, k, cc * P:(cc + 1) * P],
                                rhs=at_sp[:, k, q0:q0 + qs],
                                start=(k == 0), stop=(k == kt - 1))
                    for (ps, cc, q0, qs) in groups:
                        nc.vector.tensor_copy(g1T[:, cc, q0:q0 + qs], ps[:, :qs])
                else:
                    for cc in range(ct):
                        for (q0, qs) in nqv:
                            ps = psA.tile([P, 512], F32, tag="psA")
                            for k in range(kt):
                                nc.tensor.matmul(
                                    ps[:, :qs],
                                    lhsT=xbf[:, k, cc * P:(cc + 1) * P],
                                    rhs=at_sp[:, k, q0:q0 + qs],
                                    start=(k == 0), stop=(k == kt - 1))
                            nc.vector.tensor_copy(g1T[:, cc, q0:q0 + qs],
                                                  ps[:, :qs])
                return g1T

            def emit_p2(b, g1T):
                # phase 2: a1T = relu(W1^T @ g1T + b1)
                a1T = actp.tile([P, ct, npad], BF16, tag="act", name=f"a1T_{b}")
                for cc in range(ct):
                    # cols [n:npad] are read as phase-3 lhsT pads but never
                    # written by the trimmed quarters
                    nc.vector.memset(a1T[:, cc, n:npad], 0)
                for co in range(ct):
                    for (q0, qs) in nqv:
                        ps = psA.tile([P, 512], F32, tag="psA")
                        for ci in range(ct):
                            nc.tensor.matmul(
                                ps[:, :qs],
                                lhsT=w_sb[0][:, ci, co * P:(co + 1) * P],
                                rhs=g1T[:, ci, q0:q0 + qs],
                                start=(ci == 0), stop=(ci == ct - 1))
                        nc.scalar.activation(a1T[:, co, q0:q0 + qs], ps[:, :qs],
                                             RELU, bias=b1_sb[:, co:co + 1])
                return a1T

            def emit_p3(b, a1T):
                # phase 3: h2 = a1 @ W2 (natural layout)
                h2 = hp.tile([P, kt, c], BF16, tag="h", name=f"h2_{b}")
                for k in range(kt):
                    ps = psW.tile([P, c], F32, tag="psW")
                    for ci in range(ct):
                        nc.tensor.matmul(
                            ps[:],
                            lhsT=a1T[:, ci, k * P:(k + 1) * P],
                            rhs=w_sb[1][:, ci, :],
                            start=(ci == 0), stop=(ci == ct - 1))
                    nc.vector.tensor_copy(h2[:, k, :], ps[:])
                return h2

            def emit_p4(b, h2):
                # phase 4: a2T = relu((A_tm @ h2)^T + b2)
                a2T = actp.tile([P, ct, npad], BF16, tag="act", name=f"a2T_{b}")
                for cc in range(ct):
                    nc.vector.memset(a2T[:, cc, n:npad], 0)
                if b == 0:
                    # k-outer in two 4-bank rounds so at_tm tiles are
                    # consumed while their DMAs are still landing
                    for cc in range(ct):
                        groups = [(psA.tile([P, 512], F32, tag="psA",
                                            name=f"ps4_{cc}_{q0}"), q0, qs)
                                  for (q0, qs) in nqv]
                        for k in range(kt):
                            for (ps, q0, qs) in groups:
                                nc.tensor.matmul(
                                    ps[:, :qs],
                                    lhsT=h2[:, k, cc * P:(cc + 1) * P],
                                    rhs=at_tm[:, k, q0:q0 + qs],
                                    start=(k == 0), stop=(k == kt - 1))
                        for (ps, q0, qs) in groups:
                            nc.scalar.activation(a2T[:, cc, q0:q0 + qs],
                                                 ps[:, :qs], RELU,
                                                 bias=b2_sb[:, cc:cc + 1])
                else:
                    for cc in range(ct):
                        for (q0, qs) in nqv:
                            ps = psA.tile([P, 512], F32, tag="psA")
                            for k in range(kt):
                                nc.tensor.matmul(
                                    ps[:, :qs],
                                    lhsT=h2[:, k, cc * P:(cc + 1) * P],
                                    rhs=at_tm[:, k, q0:q0 + qs],
                                    start=(k == 0), stop=(k == kt - 1))
                            nc.scalar.activation(a2T[:, cc, q0:q0 + qs],
                                                 ps[:, :qs], RELU,
                                                 bias=b2_sb[:, cc:cc + 1])
                return a2T

            def emit_p5(b, a2T):
                # phase 5: h3 = a2 @ W3; h3[row n] = b3
                h3 = hp.tile([P, kt, c], BF16, tag="h", name=f"h3_{b}")
                for k in range(kt):
                    ps = psW.tile([P, c], F32, tag="psW")
                    for ci in range(ct):
                        nc.tensor.matmul(
                            ps[:],
                            lhsT=a2T[:, ci, k * P:(k + 1) * P],
                            rhs=w_sb[2][:, ci, :],
                            start=(ci == 0), stop=(ci == ct - 1))
                    nc.vector.tensor_copy(h3[:, k, :], ps[:])
                nc.scalar.dma_start(
                    h3[bias_part:bias_part + 1, bias_tile, :], b3_d[:, :])
                return h3

            def emit_p6(b, h3):
                # phase 6: out = relu(A_sp @ h3 + x)
                for ko in range(kt):
                    rows = min(P, n - ko * P)
                    if rows <= 0:
                        continue
                    xr = xresp.tile([P, c], F32, tag="xr")
                    nc.gpsimd.dma_start(xr[:rows, :],
                                        x_d[b, ko * P:ko * P + rows, :])
                    ps = psW.tile([P, c], F32, tag="psW")
                    for k in range(kt):
                        nc.tensor.matmul(
                            ps[:rows, :],
                            lhsT=at_sp[:, k, ko * P:ko * P + rows],
                            rhs=h3[:, k, :],
                            start=(k == 0), stop=(k == kt - 1))
                    ot = outp.tile([P, c], F32, tag="o")
                    nc.vector.tensor_add(ot[:rows, :], ps[:rows, :], xr[:rows, :])
                    nc.scalar.activation(ot[:rows, :], ot[:rows, :], RELU)
                    nc.scalar.dma_start(out_d[b, ko * P:ko * P + rows, :],
                                        ot[:rows, :])

            def emit_item(b, xbf=None, g1T=None):
                if xbf is None:
                    xbf = emit_load_x(b)
                if g1T is None:
                    g1T = emit_p1(b, xbf)
                a1T = emit_p2(b, g1T)
                h2 = emit_p3(b, a1T)
                a2T = emit_p4(b, h2)
                h3 = emit_p5(b, a2T)
                emit_p6(b, h3)

            # Emission order: item-1 phase 1 is hoisted between item-0
            # phase 1 and phase 2 so the PE has ~21us more matmul work
            # before the first at_tm use (its DMA trails at_sp).
            xbf0 = emit_load_x(0)
            g1T0 = emit_p1(0, xbf0)
            if bl > 1:
                xbf1 = emit_load_x(1)
                g1T1 = emit_p1(1, xbf1)
            emit_item(0, xbf0, g1T0)
            if bl > 1:
                emit_item(1, xbf1, g1T1)
            for b in range(2, bl):
                emit_item(b)

    nc.compile()
    return nc


def _norm_adj_T(edges, n, npad, bias_row):
    """A^T padded to [npad, npad] in bf16. AT[m, j] = A[j, m] where
    out[j] += A[j, m] * h[m]; edge (r -> c) contributes dinv[r]*dinv[c] at
    AT[r, c]. Self loops included. If bias_row, AT[n, :n] = 1 (bias fold)."""
    row = np.concatenate([edges[0], np.arange(n, dtype=np.int64)])
    col = np.concatenate([edges[1], np.arange(n, dtype=np.int64)])
    deg = np.bincount(col, minlength=n).astype(np.float32)
    dinv = np.zeros(n, np.float32)
    nz = deg > 0
    dinv[nz] = 1.0 / np.sqrt(deg[nz])
    norm = dinv[row] * dinv[col]
    at = np.zeros((npad, npad), np.float32)
    np.add.at(at, (row, col), norm)
    if bias_row:
        at[n, :n] = 1.0
    return at.astype(NP_BF16)


def _tile_rows(a, kt):
    """[kt*P, F] -> [P, kt, F] so that [p, k, :] = a[k*P + p, :]."""
    return np.ascontiguousarray(
        a.reshape(kt, P, a.shape[-1]).transpose(1, 0, 2))


_PROGRAM_CACHE = {}


def _get_program(bl, n, c):
    key = (bl, n, c)
    if key not in _PROGRAM_CACHE:
        _PROGRAM_CACHE[key] = build_program(bl, n, c)
    return _PROGRAM_CACHE[key]


def run(inputs, trace=False, n_cores=N_CORES):
    x = np.ascontiguousarray(np.asarray(inputs["x"], dtype=np.float32))
    w1 = np.asarray(inputs["W1"], np.float32)
    w2 = np.asarray(inputs["W2"], np.float32)
    w3 = np.asarray(inputs["W3"], np.float32)
    b1 = np.asarray(inputs["b1"], np.float32)
    b2 = np.asarray(inputs["b2"], np.float32)
    b3 = np.asarray(inputs["b3"], np.float32)
    e_sp = np.asarray(inputs["keypoint_line_without_temporal"]).astype(np.int64)
    e_tm = np.asarray(inputs["keypoint_line_with_temporal"]).astype(np.int64)

    b_total, n, c = x.shape
    bl = b_total // n_cores
    kt = -(-(n + 1) // P)
    npad = kt * P
    ct = c // P

    nc = _get_program(bl, n, c)

    at_sp = _tile_rows(_norm_adj_T(e_sp, n, npad, bias_row=True)[:, :n], kt)
    at_tm = _tile_rows(_norm_adj_T(e_tm, n, npad, bias_row=False)[:, :n], kt)
    shared = {
        "at_sp": at_sp,
        "at_tm": at_tm,
        "w1": _tile_rows(w1.astype(NP_BF16), ct),
        "w2": _tile_rows(w2.astype(NP_BF16), ct),
        "w3": _tile_rows(w3.astype(NP_BF16), ct),
        "b1": np.ascontiguousarray(b1.reshape(ct, P).T),
        "b2": np.ascontiguousarray(b2.reshape(ct, P).T),
        "b3": np.ascontiguousarray(b3.astype(NP_BF16)[None, :]),
    }
    in_maps = [
        {"x": np.ascontiguousarray(x[i * bl:(i + 1) * bl]), **shared}
        for i in range(n_cores)
    ]
    res = run_bass_kernel_spmd(nc, in_maps, core_ids=list(range(n_cores)),
                               trace=trace)
    out = np.concatenate([r["out"] for r in res.results], axis=0)
    return out, res


def kernel(**inputs) -> np.ndarray:
    out, _ = run(inputs, trace=False)
    return out


# revision 24
# speedup vs baseline: 1.0096x; 1.0096x over previous
"""Trainium2 Bass kernel for nn_BasicBlock (3-layer GCN block with residual).

Math (per batch item b, per conv):
    out = A @ (x @ W) + bias,  A = normalized adjacency (with self loops)
where A[c, r] = sum over edges r->c of dinv[r]*dinv[c] (dense N x N, shared
across batch and precomputed on host from the edge lists).

Block:
    a1 = relu(A_sp @ (x  @ W1) + b1)
    a2 = relu(A_tm @ (a1 @ W2) + b2)
    o3 =      A_sp @ (a2 @ W3) + b3
    out = relu(o3 + x)

On-chip layouts per item (P=128 partitions):
    natural  [n, c] : node chunks on partitions           (rhs of A-matmul /
                                                           lhsT of form-iv)
    transposed [c, n]: channel chunks on partitions        (consumed by W-matmul)

Phases per item (matmul forms; AT = A^T so AT[m, n] = A[n, m]):
    1. g1T[c,n]  = sum_m x[m,c]  * AT_sp[m,n]      (lhsT=x chunk,  rhs=AT_sp)
    2. a1T[co,n] = relu(sum_ci W1[ci,co]*g1T[ci,n] + b1)   (lhsT=W1, rhs=g1T)
    3. h2[n,c]   = sum_ci a1T[ci,n] * W2[ci,c]     (lhsT=a1T chunk, rhs=W2)
    4. a2T[c,n]  = relu(sum_m h2[m,c]*AT_tm[m,n] + b2)
    5. h3[n,c]   = sum_ci a2T[ci,n] * W3[ci,c];  h3[N,:] = b3
    6. out[n,c]  = relu(sum_m AT_sp[m,n]*h3[m,c] + x[n,c])
       (AT_sp row N is all-ones over valid cols -> adds b3 to every node;
        harmless in phase 1 because x row N is zero-padded)

All matmuls bf16 (1 cycle/row on PE) with fp32 PSUM accumulation; residual is
added in fp32 from a fresh x DMA. Batch (64) is sharded 8 items/core over the
8 cores; A matrices / weights are replicated.
"""

import sys

if "/opt/trn_rl_repo" not in sys.path:
    sys.path.insert(0, "/opt/trn_rl_repo")

import numpy as np
import ml_dtypes

import concourse.bass as bass
import concourse.bacc as bacc
import concourse.mybir as mybir
import concourse.tile as tile
from concourse.bass_utils import run_bass_kernel_spmd

P = 128
B, N, C = 64, 1700, 256
N_CORES = 8
B_LOCAL = B // N_CORES

F32 = mybir.dt.float32
BF16 = mybir.dt.bfloat16
RELU = mybir.ActivationFunctionType.Relu
NP_BF16 = ml_dtypes.bfloat16


def _quarters(total, step=512):
    return [(q, min(step, total - q)) for q in range(0, total, step)]


def build_program(bl, n, c):
    """Build the Bass/Tile program for `bl` batch items, `n` nodes, `c` chans."""
    kt = -(-(n + 1) // P)  # node chunks; >= one pad row (bias row at index n)
    npad = kt * P
    ct = c // P
    nq = _quarters(npad)

    nqv = _quarters(n)  # valid-column quarters (phases whose pads are unread)

    nc = bacc.Bacc("TRN2", target_bir_lowering=False, debug=False,
                   enable_asserts=False)

    x_d = nc.dram_tensor("x", [bl, n, c], F32, kind="ExternalInput")
    atsp_d = nc.dram_tensor("at_sp", [P, kt, n], BF16, kind="ExternalInput")
    attm_d = nc.dram_tensor("at_tm", [P, kt, n], BF16, kind="ExternalInput")
    w_d = [nc.dram_tensor(f"w{i}", [P, ct, c], BF16, kind="ExternalInput")
           for i in (1, 2, 3)]
    b1_d = nc.dram_tensor("b1", [P, ct], F32, kind="ExternalInput")
    b2_d = nc.dram_tensor("b2", [P, ct], F32, kind="ExternalInput")
    b3_d = nc.dram_tensor("b3", [1, c], BF16, kind="ExternalInput")
    out_d = nc.dram_tensor("out", [bl, n, c], F32, kind="ExternalOutput")

    with tile.TileContext(nc) as tc:
        with (
            tc.tile_pool(name="const", bufs=1) as cpool,
            tc.tile_pool(name="xin", bufs=4) as xin,
            tc.tile_pool(name="xbf", bufs=3) as xbfp,
            tc.tile_pool(name="act", bufs=4) as actp,
            tc.tile_pool(name="h", bufs=3) as hp,
            tc.tile_pool(name="xres", bufs=4) as xresp,
            tc.tile_pool(name="outp", bufs=4) as outp,
            tc.tile_pool(name="psA", bufs=4, space="PSUM") as psA,
            tc.tile_pool(name="psW", bufs=4, space="PSUM") as psW,
        ):
            # --- constants.  Ring plan: at_sp is needed first (item-0
            # phase 1 consumes tile k at ~1.4*k us), so it is split across
            # the sync+vector HWDGE rings with at_tm queued behind it on the
            # same rings; x goes on the scalar ring; xr prefetches on
            # gpsimd; out stores on scalar. ---
            w_sb = []
            for i, wd in enumerate(w_d):
                w = cpool.tile([P, ct, c], BF16, tag=f"w{i}")
                nc.scalar.dma_start(w[:], wd[:])
                w_sb.append(w)
            b1_sb = cpool.tile([P, ct], F32, tag="b1")
            b2_sb = cpool.tile([P, ct], F32, tag="b2")
            nc.scalar.dma_start(b1_sb[:], b1_d[:])
            nc.scalar.dma_start(b2_sb[:], b2_d[:])

            at_sp = cpool.tile([P, kt, n], BF16, tag="at_sp")
            at_tm = cpool.tile([P, kt, n], BF16, tag="at_tm")
            for k in range(kt):
                eng = nc.sync if k % 2 == 0 else nc.scalar
                eng.dma_start(at_sp[:, k, :], atsp_d[:, k, :])

            def emit_load_at_tm():
                # queued on the rings behind at_sp (and behind item-1's x on
                # sync) -- needed only from item-0 phase 4 (~52us in)
                for k in range(kt):
                    eng = nc.sync if k % 2 == 0 else nc.scalar
                    eng.dma_start(at_tm[:, k, :], attm_d[:, k, :])

            bias_tile = n // P      # global node index n == first pad row
            bias_part = n % P

            def emit_load_x(b):
                # load x, cast to bf16 (pad rows zero)
                x_eng = nc.gpsimd if b == 0 else nc.sync
                xbf = xbfp.tile([P, kt, c], BF16, tag="xbf", name=f"xbf_{b}")
                for k in range(kt):
                    rows = min(P, n - k * P)
                    if rows <= 0:
                        nc.vector.memset(xbf[:, k, :], 0)
                        continue
                    xf = xin.tile([P, c], F32, tag="xf", name=f"xf_{b}_{k}")
                    x_eng.dma_start(xf[:rows, :], x_d[b, k * P:k * P + rows, :])
                    if rows < P:
                        nc.vector.memset(xbf[:, k, :], 0)
                    nc.vector.tensor_copy(xbf[:rows, k, :], xf[:rows, :])
                return xbf

            def emit_p1(b, xbf):
                # phase 1: g1T = (A_sp @ x)^T
                g1T = actp.tile([P, ct, npad], BF16, tag="act", name=f"g1T_{b}")
                if b == 0:
                    # k-outer over 8 parallel PSUM banks so tile k of at_sp
                    # is consumed as soon as its DMA lands
                    groups = []
                    for cc in range(ct):
                        for qi, (q0, qs) in enumerate(nqv):
                            pool, tg = ((psA, "psA")
                                        if (cc * len(nqv) + qi) % 2 == 0
                                        else (psW, "psW"))
                            groups.append(
                                (pool.tile([P, 512], F32, tag=tg,
                                           name=f"ps1_{cc}_{qi}"), cc, q0, qs))
                    for k in range(kt):
                        for (ps, cc, q0, qs) in groups:
                            nc.tensor.matmul(
                                ps[:, :qs],
                                lhsT=xbf[:, k, cc * P:(cc + 1) * P],
                                rhs=at_sp[:, k, q0:q0 + qs],
                                start=(k == 0), stop=(k == kt - 1))
                    for (ps, cc, q0, qs) in groups:
                        nc.vector.tensor_copy(g1T[:, cc, q0:q0 + qs], ps[:, :qs])
                else:
                    for cc in range(ct):
                        for (q0, qs) in nqv:
                            ps = psA.tile([P, 512], F32, tag="psA")
                            for k in range(kt):
                                nc.tensor.matmul(
                                    ps[:, :qs],
                                    lhsT=xbf[:, k, cc * P:(cc + 1) * P],
                                    rhs=at_sp[:, k, q0:q0 + qs],
                                    start=(k == 0), stop=(k == kt - 1))
                            nc.vector.tensor_copy(g1T[:, cc, q0:q0 + qs],
                                                  ps[:, :qs])
                return g1T

            def emit_p2(b, g1T):
                # phase 2: a1T = relu(W1^T @ g1T + b1)
                a1T = actp.tile([P, ct, npad], BF16, tag="act", name=f"a1T_{b}")
                for cc in range(ct):
                    # cols [n:npad] are read as phase-3 lhsT pads but never
                    # written by the trimmed quarters
                    nc.vector.memset(a1T[:, cc, n:npad], 0)
                for co in range(ct):
                    for (q0, qs) in nqv:
                        ps = psA.tile([P, 512], F32, tag="psA")
                        for ci in range(ct):
                            nc.tensor.matmul(
                                ps[:, :qs],
                                lhsT=w_sb[0][:, ci, co * P:(co + 1) * P],
                                rhs=g1T[:, ci, q0:q0 + qs],
                                start=(ci == 0), stop=(ci == ct - 1))
                        nc.scalar.activation(a1T[:, co, q0:q0 + qs], ps[:, :qs],
                                             RELU, bias=b1_sb[:, co:co + 1])
                return a1T

            def emit_p3(b, a1T):
                # phase 3: h2 = a1 @ W2 (natural layout)
                h2 = hp.tile([P, kt, c], BF16, tag="h", name=f"h2_{b}")
                for k in range(kt):
                    ps = psW.tile([P, c], F32, tag="psW")
                    for ci in range(ct):
                        nc.tensor.matmul(
                            ps[:],
                            lhsT=a1T[:, ci, k * P:(k + 1) * P],
                            rhs=w_sb[1][:, ci, :],
                            start=(ci == 0), stop=(ci == ct - 1))
                    nc.vector.tensor_copy(h2[:, k, :], ps[:])
                return h2

            def emit_p4(b, h2):
                # phase 4: a2T = relu((A_tm @ h2)^T + b2)
                a2T = actp.tile([P, ct, npad], BF16, tag="act", name=f"a2T_{b}")
                for cc in range(ct):
                    nc.vector.memset(a2T[:, cc, n:npad], 0)
                if b == 0:
                    # k-outer in two 4-bank rounds so at_tm tiles are
                    # consumed while their DMAs are still landing
                    for cc in range(ct):
                        groups = [(psA.tile([P, 512], F32, tag="psA",
                                            name=f"ps4_{cc}_{q0}"), q0, qs)
                                  for (q0, qs) in nqv]
                        for k in range(kt):
                            for (ps, q0, qs) in groups:
                                nc.tensor.matmul(
                                    ps[:, :qs],
                                    lhsT=h2[:, k, cc * P:(cc + 1) * P],
                                    rhs=at_tm[:, k, q0:q0 + qs],
                                    start=(k == 0), stop=(k == kt - 1))
                        for (ps, q0, qs) in groups:
                            nc.scalar.activation(a2T[:, cc, q0:q0 + qs],
                                                 ps[:, :qs], RELU,
                                                 bias=b2_sb[:, cc:cc + 1])
                else:
                    for cc in range(ct):
                        for (q0, qs) in nqv:
                            ps = psA.tile([P, 512], F32, tag="psA")
                            for k in range(kt):
                                nc.tensor.matmul(
                                    ps[:, :qs],
                                    lhsT=h2[:, k, cc * P:(cc + 1) * P],
                                    rhs=at_tm[:, k, q0:q0 + qs],
                                    start=(k == 0), stop=(k == kt - 1))
                            nc.scalar.activation(a2T[:, cc, q0:q0 + qs],
                                                 ps[:, :qs], RELU,
                                                 bias=b2_sb[:, cc:cc + 1])
                return a2T

            def emit_p5(b, a2T):
                # phase 5: h3 = a2 @ W3; h3[row n] = b3
                h3 = hp.tile([P, kt, c], BF16, tag="h", name=f"h3_{b}")
                for k in range(kt):
                    ps = psW.tile([P, c], F32, tag="psW")
                    for ci in range(ct):
                        nc.tensor.matmul(
                            ps[:],
                            lhsT=a2T[:, ci, k * P:(k + 1) * P],
                            rhs=w_sb[2][:, ci, :],
                            start=(ci == 0), stop=(ci == ct - 1))
                    nc.vector.tensor_copy(h3[:, k, :], ps[:])
                nc.scalar.dma_start(
                    h3[bias_part:bias_part + 1, bias_tile, :], b3_d[:, :])
                return h3

            def emit_p6(b, h3):
                # phase 6: out = relu(A_sp @ h3 + x)
                for ko in range(kt):
                    rows = min(P, n - ko * P)
                    if rows <= 0:
                        continue
                    xr = xresp.tile([P, c], F32, tag="xr")
                    nc.gpsimd.dma_start(xr[:rows, :],
                                        x_d[b, ko * P:ko * P + rows, :])
                    ps = psW.tile([P, c], F32, tag="psW")
                    for k in range(kt):
                        nc.tensor.matmul(
                            ps[:rows, :],
                            lhsT=at_sp[:, k, ko * P:ko * P + rows],
                            rhs=h3[:, k, :],
                            start=(k == 0), stop=(k == kt - 1))
                    ot = outp.tile([P, c], F32, tag="o")
                    nc.vector.tensor_add(ot[:rows, :], ps[:rows, :], xr[:rows, :])
                    nc.scalar.activation(ot[:rows, :], ot[:rows, :], RELU)
                    nc.scalar.dma_start(out_d[b, ko * P:ko * P + rows, :],
                                        ot[:rows, :])

            def emit_item(b, xbf=None, g1T=None):
                if xbf is None:
                    xbf = emit_load_x(b)
                if g1T is None:
                    g1T = emit_p1(b, xbf)
                a1T = emit_p2(b, g1T)
                h2 = emit_p3(b, a1T)
                a2T = emit_p4(b, h2)
                h3 = emit_p5(b, a2T)
                emit_p6(b, h3)

            # Emission order: item-1 phase 1 is hoisted between item-0
            # phase 1 and phase 2 so the PE has ~21us more matmul work
            # before the first at_tm use (its DMA trails at_sp).
            xbf0 = emit_load_x(0)
            g1T0 = emit_p1(0, xbf0)
            if bl > 1:
                xbf1 = emit_load_x(1)
                emit_load_at_tm()
                g1T1 = emit_p1(1, xbf1)
            else:
                emit_load_at_tm()
            emit_item(0, xbf0, g1T0)
            if bl > 1:
                emit_item(1, xbf1, g1T1)
            for b in range(2, bl):
                emit_item(b)

    nc.compile()
    return nc


def _norm_adj_T(edges, n, npad, bias_row):
    """A^T padded to [npad, npad] in bf16. AT[m, j] = A[j, m] where
    out[j] += A[j, m] * h[m]; edge (r -> c) contributes dinv[r]*dinv[c] at
    AT[r, c]. Self loops included. If bias_row, AT[n, :n] = 1 (bias fold)."""
    row = np.concatenate([edges[0], np.arange(n, dtype=np.int64)])
    col = np.concatenate([edges[1], np.arange(n, dtype=np.int64)])
    deg = np.bincount(col, minlength=n).astype(np.float32)
    dinv = np.zeros(n, np.float32)
    nz = deg > 0
    dinv[nz] = 1.0 / np.sqrt(deg[nz])
    norm = dinv[row] * dinv[col]
    at = np.zeros((npad, npad), np.float32)
    np.add.at(at, (row, col), norm)
    if bias_row:
        at[n, :n] = 1.0
    return at.astype(NP_BF16)


def _tile_rows(a, kt):
    """[kt*P, F] -> [P, kt, F] so that [p, k, :] = a[k*P + p, :]."""
    return np.ascontiguousarray(
        a.reshape(kt, P, a.shape[-1]).transpose(1, 0, 2))


_PROGRAM_CACHE = {}


def _get_program(bl, n, c):
    key = (bl, n, c)
    if key not in _PROGRAM_CACHE:
        _PROGRAM_CACHE[key] = build_program(bl, n, c)
    return _PROGRAM_CACHE[key]


def run(inputs, trace=False, n_cores=N_CORES):
    x = np.ascontiguousarray(np.asarray(inputs["x"], dtype=np.float32))
    w1 = np.asarray(inputs["W1"], np.float32)
    w2 = np.asarray(inputs["W2"], np.float32)
    w3 = np.asarray(inputs["W3"], np.float32)
    b1 = np.asarray(inputs["b1"], np.float32)
    b2 = np.asarray(inputs["b2"], np.float32)
    b3 = np.asarray(inputs["b3"], np.float32)
    e_sp = np.asarray(inputs["keypoint_line_without_temporal"]).astype(np.int64)
    e_tm = np.asarray(inputs["keypoint_line_with_temporal"]).astype(np.int64)

    b_total, n, c = x.shape
    bl = b_total // n_cores
    kt = -(-(n + 1) // P)
    npad = kt * P
    ct = c // P

    nc = _get_program(bl, n, c)

    at_sp = _tile_rows(_norm_adj_T(e_sp, n, npad, bias_row=True)[:, :n], kt)
    at_tm = _tile_rows(_norm_adj_T(e_tm, n, npad, bias_row=False)[:, :n], kt)
    shared = {
        "at_sp": at_sp,
        "at_tm": at_tm,
        "w1": _tile_rows(w1.astype(NP_BF16), ct),
        "w2": _tile_rows(w2.astype(NP_BF16), ct),
        "w3": _tile_rows(w3.astype(NP_BF16), ct),
        "b1": np.ascontiguousarray(b1.reshape(ct, P).T),
        "b2": np.ascontiguousarray(b2.reshape(ct, P).T),
        "b3": np.ascontiguousarray(b3.astype(NP_BF16)[None, :]),
    }
    in_maps = [
        {"x": np.ascontiguousarray(x[i * bl:(i + 1) * bl]), **shared}
        for i in range(n_cores)
    ]
    res = run_bass_kernel_spmd(nc, in_maps, core_ids=list(range(n_cores)),
                               trace=trace)
    out = np.concatenate([r["out"] for r in res.results], axis=0)
    return out, res


def kernel(**inputs) -> np.ndarray:
    out, _ = run(inputs, trace=False)
    return out


# revision 26
# speedup vs baseline: 1.0552x; 1.0452x over previous
"""Trainium2 Bass kernel for nn_BasicBlock (3-layer GCN block with residual).

Math (per batch item b, per conv):
    out = A @ (x @ W) + bias,  A = normalized adjacency (with self loops)
where A[c, r] = sum over edges r->c of dinv[r]*dinv[c] (dense N x N, shared
across batch and precomputed on host from the edge lists).

Block:
    a1 = relu(A_sp @ (x  @ W1) + b1)
    a2 = relu(A_tm @ (a1 @ W2) + b2)
    o3 =      A_sp @ (a2 @ W3) + b3
    out = relu(o3 + x)

On-chip layouts per item (P=128 partitions):
    natural  [n, c] : node chunks on partitions           (rhs of A-matmul /
                                                           lhsT of form-iv)
    transposed [c, n]: channel chunks on partitions        (consumed by W-matmul)

Phases per item (matmul forms; AT = A^T so AT[m, n] = A[n, m]):
    1. g1T[c,n]  = sum_m x[m,c]  * AT_sp[m,n]      (lhsT=x chunk,  rhs=AT_sp)
    2. a1T[co,n] = relu(sum_ci W1[ci,co]*g1T[ci,n] + b1)   (lhsT=W1, rhs=g1T)
    3. h2[n,c]   = sum_ci a1T[ci,n] * W2[ci,c]     (lhsT=a1T chunk, rhs=W2)
    4. a2T[c,n]  = relu(sum_m h2[m,c]*AT_tm[m,n] + b2)
    5. h3[n,c]   = sum_ci a2T[ci,n] * W3[ci,c];  h3[N,:] = b3
    6. out[n,c]  = relu(sum_m AT_sp[m,n]*h3[m,c] + x[n,c])
       (AT_sp row N is all-ones over valid cols -> adds b3 to every node;
        harmless in phase 1 because x row N is zero-padded)

All matmuls bf16 (1 cycle/row on PE) with fp32 PSUM accumulation; x arrives
pre-cast to bf16 from the host and doubles as the phase-6 residual. Batch
(64) is sharded 8 items/core over the 8 cores; A/W/b are replicated.
"""

import sys

if "/opt/trn_rl_repo" not in sys.path:
    sys.path.insert(0, "/opt/trn_rl_repo")

import numpy as np
import ml_dtypes

import concourse.bass as bass
import concourse.bacc as bacc
import concourse.mybir as mybir
import concourse.tile as tile
from concourse.bass_utils import run_bass_kernel_spmd

P = 128
B, N, C = 64, 1700, 256
N_CORES = 8
B_LOCAL = B // N_CORES

F32 = mybir.dt.float32
BF16 = mybir.dt.bfloat16
RELU = mybir.ActivationFunctionType.Relu
NP_BF16 = ml_dtypes.bfloat16


def _quarters(total, step=512):
    return [(q, min(step, total - q)) for q in range(0, total, step)]


def build_program(bl, n, c):
    """Build the Bass/Tile program for `bl` batch items, `n` nodes, `c` chans."""
    kt = -(-(n + 1) // P)  # node chunks; >= one pad row (bias row at index n)
    npad = kt * P
    ct = c // P
    nq = _quarters(npad)

    nqv = _quarters(n)  # valid-column quarters (phases whose pads are unread)

    nc = bacc.Bacc("TRN2", target_bir_lowering=False, debug=False,
                   enable_asserts=False)

    x_d = nc.dram_tensor("x", [bl, n, c], BF16, kind="ExternalInput")
    atsp_d = nc.dram_tensor("at_sp", [P, kt, n], BF16, kind="ExternalInput")
    attm_d = nc.dram_tensor("at_tm", [P, kt, n], BF16, kind="ExternalInput")
    w_d = [nc.dram_tensor(f"w{i}", [P, ct, c], BF16, kind="ExternalInput")
           for i in (1, 2, 3)]
    b1_d = nc.dram_tensor("b1", [P, ct], F32, kind="ExternalInput")
    b2_d = nc.dram_tensor("b2", [P, ct], F32, kind="ExternalInput")
    b3_d = nc.dram_tensor("b3", [1, c], BF16, kind="ExternalInput")
    out_d = nc.dram_tensor("out", [bl, n, c], F32, kind="ExternalOutput")

    with tile.TileContext(nc) as tc:
        with (
            tc.tile_pool(name="const", bufs=1) as cpool,
            tc.tile_pool(name="xbf", bufs=4) as xbfp,
            tc.tile_pool(name="act", bufs=4) as actp,
            tc.tile_pool(name="h", bufs=3) as hp,
            tc.tile_pool(name="outp", bufs=4) as outp,
            tc.tile_pool(name="psA", bufs=4, space="PSUM") as psA,
            tc.tile_pool(name="psW", bufs=4, space="PSUM") as psW,
        ):
            # --- constants.  Ring plan: at_sp is needed first (item-0
            # phase 1 consumes tile k at ~1.4*k us), so it is split across
            # the sync+vector HWDGE rings with at_tm queued behind it on the
            # same rings; x goes on the scalar ring; xr prefetches on
            # gpsimd; out stores on scalar. ---
            w_sb = []
            for i, wd in enumerate(w_d):
                w = cpool.tile([P, ct, c], BF16, tag=f"w{i}")
                nc.scalar.dma_start(w[:], wd[:])
                w_sb.append(w)
            b1_sb = cpool.tile([P, ct], F32, tag="b1")
            b2_sb = cpool.tile([P, ct], F32, tag="b2")
            nc.scalar.dma_start(b1_sb[:], b1_d[:])
            nc.scalar.dma_start(b2_sb[:], b2_d[:])

            at_sp = cpool.tile([P, kt, n], BF16, tag="at_sp")
            at_tm = cpool.tile([P, kt, n], BF16, tag="at_tm")
            for k in range(kt):
                eng = nc.sync if k % 2 == 0 else nc.scalar
                eng.dma_start(at_sp[:, k, :], atsp_d[:, k, :])

            def emit_load_at_tm():
                # queued on the rings behind at_sp (and behind item-1's x on
                # sync) -- needed only from item-0 phase 4 (~52us in)
                for k in range(kt):
                    eng = nc.sync if k % 2 == 0 else nc.scalar
                    eng.dma_start(at_tm[:, k, :], attm_d[:, k, :])

            bias_tile = n // P      # global node index n == first pad row
            bias_part = n % P

            def emit_load_x(b, eng=None):
                # x arrives pre-cast bf16 from the host; DMA straight into
                # the padded [P, kt, c] tile (pad rows zeroed)
                x_eng = eng if eng is not None else (
                    nc.gpsimd if b <= 1 else nc.sync)
                xbf = xbfp.tile([P, kt, c], BF16, tag="xbf", name=f"xbf_{b}")
                for k in range(kt):
                    rows = min(P, n - k * P)
                    if rows < P:
                        nc.vector.memset(xbf[:, k, :], 0)
                    if rows > 0:
                        x_eng.dma_start(xbf[:rows, k, :],
                                        x_d[b, k * P:k * P + rows, :])
                return xbf

            def emit_p1(b, xbf):
                # phase 1: g1T = (A_sp @ x)^T
                g1T = actp.tile([P, ct, npad], BF16, tag="act", name=f"g1T_{b}")
                if b == 0:
                    # k-outer over 8 parallel PSUM banks so tile k of at_sp
                    # is consumed as soon as its DMA lands
                    groups = []
                    for cc in range(ct):
                        for qi, (q0, qs) in enumerate(nqv):
                            pool, tg = ((psA, "psA")
                                        if (cc * len(nqv) + qi) % 2 == 0
                                        else (psW, "psW"))
                            groups.append(
                                (pool.tile([P, 512], F32, tag=tg,
                                           name=f"ps1_{cc}_{qi}"), cc, q0, qs))
                    for k in range(kt):
                        for (ps, cc, q0, qs) in groups:
                            nc.tensor.matmul(
                                ps[:, :qs],
                                lhsT=xbf[:, k, cc * P:(cc + 1) * P],
                                rhs=at_sp[:, k, q0:q0 + qs],
                                start=(k == 0), stop=(k == kt - 1))
                    for (ps, cc, q0, qs) in groups:
                        nc.vector.tensor_copy(g1T[:, cc, q0:q0 + qs], ps[:, :qs])
                else:
                    for cc in range(ct):
                        for (q0, qs) in nqv:
                            ps = psA.tile([P, 512], F32, tag="psA")
                            for k in range(kt):
                                nc.tensor.matmul(
                                    ps[:, :qs],
                                    lhsT=xbf[:, k, cc * P:(cc + 1) * P],
                                    rhs=at_sp[:, k, q0:q0 + qs],
                                    start=(k == 0), stop=(k == kt - 1))
                            nc.vector.tensor_copy(g1T[:, cc, q0:q0 + qs],
                                                  ps[:, :qs])
                return g1T

            def emit_p2(b, g1T):
                # phase 2: a1T = relu(W1^T @ g1T + b1)
                a1T = actp.tile([P, ct, npad], BF16, tag="act", name=f"a1T_{b}")
                for cc in range(ct):
                    # cols [n:npad] are read as phase-3 lhsT pads but never
                    # written by the trimmed quarters
                    nc.vector.memset(a1T[:, cc, n:npad], 0)
                for co in range(ct):
                    for (q0, qs) in nqv:
                        ps = psA.tile([P, 512], F32, tag="psA")
                        for ci in range(ct):
                            nc.tensor.matmul(
                                ps[:, :qs],
                                lhsT=w_sb[0][:, ci, co * P:(co + 1) * P],
                                rhs=g1T[:, ci, q0:q0 + qs],
                                start=(ci == 0), stop=(ci == ct - 1))
                        nc.scalar.activation(a1T[:, co, q0:q0 + qs], ps[:, :qs],
                                             RELU, bias=b1_sb[:, co:co + 1])
                return a1T

            def emit_p3(b, a1T):
                # phase 3: h2 = a1 @ W2 (natural layout)
                h2 = hp.tile([P, kt, c], BF16, tag="h", name=f"h2_{b}")
                for k in range(kt):
                    ps = psW.tile([P, c], F32, tag="psW")
                    for ci in range(ct):
                        nc.tensor.matmul(
                            ps[:],
                            lhsT=a1T[:, ci, k * P:(k + 1) * P],
                            rhs=w_sb[1][:, ci, :],
                            start=(ci == 0), stop=(ci == ct - 1))
                    nc.vector.tensor_copy(h2[:, k, :], ps[:])
                return h2

            def emit_p4(b, h2):
                # phase 4: a2T = relu((A_tm @ h2)^T + b2)
                a2T = actp.tile([P, ct, npad], BF16, tag="act", name=f"a2T_{b}")
                for cc in range(ct):
                    nc.vector.memset(a2T[:, cc, n:npad], 0)
                if b == 0:
                    # k-outer in two 4-bank rounds so at_tm tiles are
                    # consumed while their DMAs are still landing
                    for cc in range(ct):
                        groups = [(psA.tile([P, 512], F32, tag="psA",
                                            name=f"ps4_{cc}_{q0}"), q0, qs)
                                  for (q0, qs) in nqv]
                        for k in range(kt):
                            for (ps, q0, qs) in groups:
                                nc.tensor.matmul(
                                    ps[:, :qs],
                                    lhsT=h2[:, k, cc * P:(cc + 1) * P],
                                    rhs=at_tm[:, k, q0:q0 + qs],
                                    start=(k == 0), stop=(k == kt - 1))
                        for (ps, q0, qs) in groups:
                            nc.scalar.activation(a2T[:, cc, q0:q0 + qs],
                                                 ps[:, :qs], RELU,
                                                 bias=b2_sb[:, cc:cc + 1])
                else:
                    for cc in range(ct):
                        for (q0, qs) in nqv:
                            ps = psA.tile([P, 512], F32, tag="psA")
                            for k in range(kt):
                                nc.tensor.matmul(
                                    ps[:, :qs],
                                    lhsT=h2[:, k, cc * P:(cc + 1) * P],
                                    rhs=at_tm[:, k, q0:q0 + qs],
                                    start=(k == 0), stop=(k == kt - 1))
                            nc.scalar.activation(a2T[:, cc, q0:q0 + qs],
                                                 ps[:, :qs], RELU,
                                                 bias=b2_sb[:, cc:cc + 1])
                return a2T

            def emit_p5(b, a2T):
                # phase 5: h3 = a2 @ W3; h3[row n] = b3
                h3 = hp.tile([P, kt, c], BF16, tag="h", name=f"h3_{b}")
                for k in range(kt):
                    ps = psW.tile([P, c], F32, tag="psW")
                    for ci in range(ct):
                        nc.tensor.matmul(
                            ps[:],
                            lhsT=a2T[:, ci, k * P:(k + 1) * P],
                            rhs=w_sb[2][:, ci, :],
                            start=(ci == 0), stop=(ci == ct - 1))
                    nc.vector.tensor_copy(h3[:, k, :], ps[:])
                nc.scalar.dma_start(
                    h3[bias_part:bias_part + 1, bias_tile, :], b3_d[:, :])
                return h3

            def emit_p6(b, xbf, h3):
                # phase 6: out = relu(A_sp @ h3 + x), residual from the
                # resident bf16 x tile
                for ko in range(kt):
                    rows = min(P, n - ko * P)
                    if rows <= 0:
                        continue
                    ps = psW.tile([P, c], F32, tag="psW")
                    for k in range(kt):
                        nc.tensor.matmul(
                            ps[:rows, :],
                            lhsT=at_sp[:, k, ko * P:ko * P + rows],
                            rhs=h3[:, k, :],
                            start=(k == 0), stop=(k == kt - 1))
                    ot = outp.tile([P, c], F32, tag="o")
                    nc.vector.tensor_add(ot[:rows, :], ps[:rows, :],
                                         xbf[:rows, ko, :])
                    nc.scalar.activation(ot[:rows, :], ot[:rows, :], RELU)
                    nc.scalar.dma_start(out_d[b, ko * P:ko * P + rows, :],
                                        ot[:rows, :])

            def emit_item(b, xbf=None, g1T=None):
                if xbf is None:
                    xbf = emit_load_x(b)
                if g1T is None:
                    g1T = emit_p1(b, xbf)
                a1T = emit_p2(b, g1T)
                h2 = emit_p3(b, a1T)
                a2T = emit_p4(b, h2)
                h3 = emit_p5(b, a2T)
                emit_p6(b, xbf, h3)

            # Emission order: item-1 phase 1 is hoisted between item-0
            # phase 1 and phase 2 so the PE has ~21us more matmul work
            # before the first at_tm use (its DMA trails at_sp).
            xbf0 = emit_load_x(0)
            g1T0 = emit_p1(0, xbf0)
            if bl > 1:
                xbf1 = emit_load_x(1)
                emit_load_at_tm()
                g1T1 = emit_p1(1, xbf1)
            else:
                emit_load_at_tm()
            emit_item(0, xbf0, g1T0)
            if bl > 1:
                emit_item(1, xbf1, g1T1)
            for b in range(2, bl):
                emit_item(b)

    nc.compile()
    return nc


def _norm_adj_T(edges, n, npad, bias_row):
    """A^T padded to [npad, npad] in bf16. AT[m, j] = A[j, m] where
    out[j] += A[j, m] * h[m]; edge (r -> c) contributes dinv[r]*dinv[c] at
    AT[r, c]. Self loops included. If bias_row, AT[n, :n] = 1 (bias fold)."""
    row = np.concatenate([edges[0], np.arange(n, dtype=np.int64)])
    col = np.concatenate([edges[1], np.arange(n, dtype=np.int64)])
    deg = np.bincount(col, minlength=n).astype(np.float32)
    dinv = np.zeros(n, np.float32)
    nz = deg > 0
    dinv[nz] = 1.0 / np.sqrt(deg[nz])
    norm = dinv[row] * dinv[col]
    at = np.zeros((npad, npad), np.float32)
    np.add.at(at, (row, col), norm)
    if bias_row:
        at[n, :n] = 1.0
    return at.astype(NP_BF16)


def _tile_rows(a, kt):
    """[kt*P, F] -> [P, kt, F] so that [p, k, :] = a[k*P + p, :]."""
    return np.ascontiguousarray(
        a.reshape(kt, P, a.shape[-1]).transpose(1, 0, 2))


_PROGRAM_CACHE = {}


def _get_program(bl, n, c):
    key = (bl, n, c)
    if key not in _PROGRAM_CACHE:
        _PROGRAM_CACHE[key] = build_program(bl, n, c)
    return _PROGRAM_CACHE[key]


def run(inputs, trace=False, n_cores=N_CORES):
    x = np.asarray(inputs["x"], dtype=np.float32).astype(NP_BF16)
    w1 = np.asarray(inputs["W1"], np.float32)
    w2 = np.asarray(inputs["W2"], np.float32)
    w3 = np.asarray(inputs["W3"], np.float32)
    b1 = np.asarray(inputs["b1"], np.float32)
    b2 = np.asarray(inputs["b2"], np.float32)
    b3 = np.asarray(inputs["b3"], np.float32)
    e_sp = np.asarray(inputs["keypoint_line_without_temporal"]).astype(np.int64)
    e_tm = np.asarray(inputs["keypoint_line_with_temporal"]).astype(np.int64)

    b_total, n, c = x.shape
    bl = b_total // n_cores
    kt = -(-(n + 1) // P)
    npad = kt * P
    ct = c // P

    nc = _get_program(bl, n, c)

    at_sp = _tile_rows(_norm_adj_T(e_sp, n, npad, bias_row=True)[:, :n], kt)
    at_tm = _tile_rows(_norm_adj_T(e_tm, n, npad, bias_row=False)[:, :n], kt)
    shared = {
        "at_sp": at_sp,
        "at_tm": at_tm,
        "w1": _tile_rows(w1.astype(NP_BF16), ct),
        "w2": _tile_rows(w2.astype(NP_BF16), ct),
        "w3": _tile_rows(w3.astype(NP_BF16), ct),
        "b1": np.ascontiguousarray(b1.reshape(ct, P).T),
        "b2": np.ascontiguousarray(b2.reshape(ct, P).T),
        "b3": np.ascontiguousarray(b3.astype(NP_BF16)[None, :]),
    }
    in_maps = [
        {"x": np.ascontiguousarray(x[i * bl:(i + 1) * bl]), **shared}
        for i in range(n_cores)
    ]
    res = run_bass_kernel_spmd(nc, in_maps, core_ids=list(range(n_cores)),
                               trace=trace)
    out = np.concatenate([r["out"] for r in res.results], axis=0)
    return out, res


def kernel(**inputs) -> np.ndarray:
    out, _ = run(inputs, trace=False)
    return out


# revision 27
# speedup vs baseline: 1.0552x; 1.0000x over previous
"""Trainium2 Bass kernel for nn_BasicBlock (3-layer GCN block with residual).

Math (per batch item b, per conv):
    out = A @ (x @ W) + bias,  A = normalized adjacency (with self loops)
where A[c, r] = sum over edges r->c of dinv[r]*dinv[c] (dense N x N, shared
across batch and precomputed on host from the edge lists).

Block:
    a1 = relu(A_sp @ (x  @ W1) + b1)
    a2 = relu(A_tm @ (a1 @ W2) + b2)
    o3 =      A_sp @ (a2 @ W3) + b3
    out = relu(o3 + x)

On-chip layouts per item (P=128 partitions):
    natural  [n, c] : node chunks on partitions           (rhs of A-matmul /
                                                           lhsT of form-iv)
    transposed [c, n]: channel chunks on partitions        (consumed by W-matmul)

Phases per item (matmul forms; AT = A^T so AT[m, n] = A[n, m]):
    1. g1T[c,n]  = sum_m x[m,c]  * AT_sp[m,n]      (lhsT=x chunk,  rhs=AT_sp)
    2. a1T[co,n] = relu(sum_ci W1[ci,co]*g1T[ci,n] + b1)   (lhsT=W1, rhs=g1T)
    3. h2[n,c]   = sum_ci a1T[ci,n] * W2[ci,c]     (lhsT=a1T chunk, rhs=W2)
    4. a2T[c,n]  = relu(sum_m h2[m,c]*AT_tm[m,n] + b2)
    5. h3[n,c]   = sum_ci a2T[ci,n] * W3[ci,c];  h3[N,:] = b3
    6. out[n,c]  = relu(sum_m AT_sp[m,n]*h3[m,c] + x[n,c])
       (AT_sp row N is all-ones over valid cols -> adds b3 to every node;
        harmless in phase 1 because x row N is zero-padded)

All matmuls bf16 (1 cycle/row on PE) with fp32 PSUM accumulation; x arrives
pre-cast to bf16 from the host and doubles as the phase-6 residual. Batch
(64) is sharded 8 items/core over the 8 cores; A/W/b are replicated.
"""

import sys

if "/opt/trn_rl_repo" not in sys.path:
    sys.path.insert(0, "/opt/trn_rl_repo")

import numpy as np
import ml_dtypes

import concourse.bass as bass
import concourse.bacc as bacc
import concourse.mybir as mybir
import concourse.tile as tile
from concourse.bass_utils import run_bass_kernel_spmd

P = 128
B, N, C = 64, 1700, 256
N_CORES = 8
B_LOCAL = B // N_CORES

F32 = mybir.dt.float32
BF16 = mybir.dt.bfloat16
RELU = mybir.ActivationFunctionType.Relu
NP_BF16 = ml_dtypes.bfloat16


def _quarters(total, step=512):
    return [(q, min(step, total - q)) for q in range(0, total, step)]


def build_program(bl, n, c):
    """Build the Bass/Tile program for `bl` batch items, `n` nodes, `c` chans."""
    kt = -(-(n + 1) // P)  # node chunks; >= one pad row (bias row at index n)
    npad = kt * P
    ct = c // P
    nq = _quarters(npad)

    nqv = _quarters(n)  # valid-column quarters (phases whose pads are unread)

    nc = bacc.Bacc("TRN2", target_bir_lowering=False, debug=False,
                   enable_asserts=False)

    x_d = nc.dram_tensor("x", [bl, n, c], BF16, kind="ExternalInput")
    atsp_d = nc.dram_tensor("at_sp", [P, kt, n], BF16, kind="ExternalInput")
    attm_d = nc.dram_tensor("at_tm", [P, kt, n], BF16, kind="ExternalInput")
    w_d = [nc.dram_tensor(f"w{i}", [P, ct, c], BF16, kind="ExternalInput")
           for i in (1, 2, 3)]
    b1_d = nc.dram_tensor("b1", [P, ct], F32, kind="ExternalInput")
    b2_d = nc.dram_tensor("b2", [P, ct], F32, kind="ExternalInput")
    b3_d = nc.dram_tensor("b3", [1, c], BF16, kind="ExternalInput")
    out_d = nc.dram_tensor("out", [bl, n, c], F32, kind="ExternalOutput")

    with tile.TileContext(nc) as tc:
        with (
            tc.tile_pool(name="const", bufs=1) as cpool,
            tc.tile_pool(name="xbf", bufs=4) as xbfp,
            tc.tile_pool(name="act", bufs=4) as actp,
            tc.tile_pool(name="h", bufs=3) as hp,
            tc.tile_pool(name="outp", bufs=4) as outp,
            tc.tile_pool(name="psA", bufs=4, space="PSUM") as psA,
            tc.tile_pool(name="psW", bufs=4, space="PSUM") as psW,
        ):
            # --- constants.  Ring plan: at_sp is needed first (item-0
            # phase 1 consumes tile k at ~1.4*k us), so it is split across
            # the sync+vector HWDGE rings with at_tm queued behind it on the
            # same rings; x goes on the scalar ring; xr prefetches on
            # gpsimd; out stores on scalar. ---
            at_sp = cpool.tile([P, kt, n], BF16, tag="at_sp")
            at_tm = cpool.tile([P, kt, n], BF16, tag="at_tm")
            nh = n // 2
            for k in range(kt):
                # split every tile across both HWDGE rings so tile k
                # completes at ~1.2*(k+1) us, tracking PE consumption
                nc.sync.dma_start(at_sp[:, k, :nh], atsp_d[:, k, :nh])
                nc.scalar.dma_start(at_sp[:, k, nh:], atsp_d[:, k, nh:])

            w_sb = []
            for i, wd in enumerate(w_d):
                w = cpool.tile([P, ct, c], BF16, tag=f"w{i}")
                nc.scalar.dma_start(w[:], wd[:])
                w_sb.append(w)
            b1_sb = cpool.tile([P, ct], F32, tag="b1")
            b2_sb = cpool.tile([P, ct], F32, tag="b2")
            nc.scalar.dma_start(b1_sb[:], b1_d[:])
            nc.scalar.dma_start(b2_sb[:], b2_d[:])

            def emit_load_at_tm():
                # queued on the rings behind at_sp (and behind item-1's x on
                # sync) -- needed only from item-0 phase 4 (~52us in)
                for k in range(kt):
                    nc.sync.dma_start(at_tm[:, k, :nh], attm_d[:, k, :nh])
                    nc.scalar.dma_start(at_tm[:, k, nh:], attm_d[:, k, nh:])

            bias_tile = n // P      # global node index n == first pad row
            bias_part = n % P

            def emit_load_x(b, eng=None):
                # x arrives pre-cast bf16 from the host; DMA straight into
                # the padded [P, kt, c] tile (pad rows zeroed)
                x_eng = eng if eng is not None else (
                    nc.gpsimd if b <= 1 else nc.sync)
                xbf = xbfp.tile([P, kt, c], BF16, tag="xbf", name=f"xbf_{b}")
                for k in range(kt):
                    rows = min(P, n - k * P)
                    if rows < P:
                        nc.vector.memset(xbf[:, k, :], 0)
                    if rows > 0:
                        x_eng.dma_start(xbf[:rows, k, :],
                                        x_d[b, k * P:k * P + rows, :])
                return xbf

            def emit_p1(b, xbf):
                # phase 1: g1T = (A_sp @ x)^T
                g1T = actp.tile([P, ct, npad], BF16, tag="act", name=f"g1T_{b}")
                if b == 0:
                    # k-outer over 8 parallel PSUM banks so tile k of at_sp
                    # is consumed as soon as its DMA lands
                    groups = []
                    for cc in range(ct):
                        for qi, (q0, qs) in enumerate(nqv):
                            pool, tg = ((psA, "psA")
                                        if (cc * len(nqv) + qi) % 2 == 0
                                        else (psW, "psW"))
                            groups.append(
                                (pool.tile([P, 512], F32, tag=tg,
                                           name=f"ps1_{cc}_{qi}"), cc, q0, qs))
                    for k in range(kt):
                        for (ps, cc, q0, qs) in groups:
                            nc.tensor.matmul(
                                ps[:, :qs],
                                lhsT=xbf[:, k, cc * P:(cc + 1) * P],
                                rhs=at_sp[:, k, q0:q0 + qs],
                                start=(k == 0), stop=(k == kt - 1))
                    for (ps, cc, q0, qs) in groups:
                        nc.vector.tensor_copy(g1T[:, cc, q0:q0 + qs], ps[:, :qs])
                else:
                    for cc in range(ct):
                        for (q0, qs) in nqv:
                            ps = psA.tile([P, 512], F32, tag="psA")
                            for k in range(kt):
                                nc.tensor.matmul(
                                    ps[:, :qs],
                                    lhsT=xbf[:, k, cc * P:(cc + 1) * P],
                                    rhs=at_sp[:, k, q0:q0 + qs],
                                    start=(k == 0), stop=(k == kt - 1))
                            nc.vector.tensor_copy(g1T[:, cc, q0:q0 + qs],
                                                  ps[:, :qs])
                return g1T

            def emit_p2(b, g1T):
                # phase 2: a1T = relu(W1^T @ g1T + b1)
                a1T = actp.tile([P, ct, npad], BF16, tag="act", name=f"a1T_{b}")
                for cc in range(ct):
                    # cols [n:npad] are read as phase-3 lhsT pads but never
                    # written by the trimmed quarters
                    nc.vector.memset(a1T[:, cc, n:npad], 0)
                for co in range(ct):
                    for (q0, qs) in nqv:
                        ps = psA.tile([P, 512], F32, tag="psA")
                        for ci in range(ct):
                            nc.tensor.matmul(
                                ps[:, :qs],
                                lhsT=w_sb[0][:, ci, co * P:(co + 1) * P],
                                rhs=g1T[:, ci, q0:q0 + qs],
                                start=(ci == 0), stop=(ci == ct - 1))
                        nc.scalar.activation(a1T[:, co, q0:q0 + qs], ps[:, :qs],
                                             RELU, bias=b1_sb[:, co:co + 1])
                return a1T

            def emit_p3(b, a1T):
                # phase 3: h2 = a1 @ W2 (natural layout)
                h2 = hp.tile([P, kt, c], BF16, tag="h", name=f"h2_{b}")
                for k in range(kt):
                    ps = psW.tile([P, c], F32, tag="psW")
                    for ci in range(ct):
                        nc.tensor.matmul(
                            ps[:],
                            lhsT=a1T[:, ci, k * P:(k + 1) * P],
                            rhs=w_sb[1][:, ci, :],
                            start=(ci == 0), stop=(ci == ct - 1))
                    nc.vector.tensor_copy(h2[:, k, :], ps[:])
                return h2

            def emit_p4(b, h2):
                # phase 4: a2T = relu((A_tm @ h2)^T + b2)
                a2T = actp.tile([P, ct, npad], BF16, tag="act", name=f"a2T_{b}")
                for cc in range(ct):
                    nc.vector.memset(a2T[:, cc, n:npad], 0)
                if b == 0:
                    # k-outer in two 4-bank rounds so at_tm tiles are
                    # consumed while their DMAs are still landing
                    for cc in range(ct):
                        groups = [(psA.tile([P, 512], F32, tag="psA",
                                            name=f"ps4_{cc}_{q0}"), q0, qs)
                                  for (q0, qs) in nqv]
                        for k in range(kt):
                            for (ps, q0, qs) in groups:
                                nc.tensor.matmul(
                                    ps[:, :qs],
                                    lhsT=h2[:, k, cc * P:(cc + 1) * P],
                                    rhs=at_tm[:, k, q0:q0 + qs],
                                    start=(k == 0), stop=(k == kt - 1))
                        for (ps, q0, qs) in groups:
                            nc.scalar.activation(a2T[:, cc, q0:q0 + qs],
                                                 ps[:, :qs], RELU,
                                                 bias=b2_sb[:, cc:cc + 1])
                else:
                    for cc in range(ct):
                        for (q0, qs) in nqv:
                            ps = psA.tile([P, 512], F32, tag="psA")
                            for k in range(kt):
                                nc.tensor.matmul(
                                    ps[:, :qs],
                                    lhsT=h2[:, k, cc * P:(cc + 1) * P],
                                    rhs=at_tm[:, k, q0:q0 + qs],
                                    start=(k == 0), stop=(k == kt - 1))
                            nc.scalar.activation(a2T[:, cc, q0:q0 + qs],
                                                 ps[:, :qs], RELU,
                                                 bias=b2_sb[:, cc:cc + 1])
                return a2T

            def emit_p5(b, a2T):
                # phase 5: h3 = a2 @ W3; h3[row n] = b3
                h3 = hp.tile([P, kt, c], BF16, tag="h", name=f"h3_{b}")
                for k in range(kt):
                    ps = psW.tile([P, c], F32, tag="psW")
                    for ci in range(ct):
                        nc.tensor.matmul(
                            ps[:],
                            lhsT=a2T[:, ci, k * P:(k + 1) * P],
                            rhs=w_sb[2][:, ci, :],
                            start=(ci == 0), stop=(ci == ct - 1))
                    nc.vector.tensor_copy(h3[:, k, :], ps[:])
                nc.scalar.dma_start(
                    h3[bias_part:bias_part + 1, bias_tile, :], b3_d[:, :])
                return h3

            def emit_p6(b, xbf, h3):
                # phase 6: out = relu(A_sp @ h3 + x), residual from the
                # resident bf16 x tile
                for ko in range(kt):
                    rows = min(P, n - ko * P)
                    if rows <= 0:
                        continue
                    ps = psW.tile([P, c], F32, tag="psW")
                    for k in range(kt):
                        nc.tensor.matmul(
                            ps[:rows, :],
                            lhsT=at_sp[:, k, ko * P:ko * P + rows],
                            rhs=h3[:, k, :],
                            start=(k == 0), stop=(k == kt - 1))
                    ot = outp.tile([P, c], F32, tag="o")
                    nc.vector.tensor_add(ot[:rows, :], ps[:rows, :],
                                         xbf[:rows, ko, :])
                    nc.scalar.activation(ot[:rows, :], ot[:rows, :], RELU)
                    nc.scalar.dma_start(out_d[b, ko * P:ko * P + rows, :],
                                        ot[:rows, :])

            def emit_item(b, xbf=None, g1T=None):
                if xbf is None:
                    xbf = emit_load_x(b)
                if g1T is None:
                    g1T = emit_p1(b, xbf)
                a1T = emit_p2(b, g1T)
                h2 = emit_p3(b, a1T)
                a2T = emit_p4(b, h2)
                h3 = emit_p5(b, a2T)
                emit_p6(b, xbf, h3)

            # Emission order: item-1 phase 1 is hoisted between item-0
            # phase 1 and phase 2 so the PE has ~21us more matmul work
            # before the first at_tm use (its DMA trails at_sp).
            xbf0 = emit_load_x(0)
            g1T0 = emit_p1(0, xbf0)
            if bl > 1:
                xbf1 = emit_load_x(1)
                emit_load_at_tm()
                g1T1 = emit_p1(1, xbf1)
            else:
                emit_load_at_tm()
            emit_item(0, xbf0, g1T0)
            if bl > 1:
                emit_item(1, xbf1, g1T1)
            for b in range(2, bl):
                emit_item(b)

    nc.compile()
    return nc


def _norm_adj_T(edges, n, npad, bias_row):
    """A^T padded to [npad, npad] in bf16. AT[m, j] = A[j, m] where
    out[j] += A[j, m] * h[m]; edge (r -> c) contributes dinv[r]*dinv[c] at
    AT[r, c]. Self loops included. If bias_row, AT[n, :n] = 1 (bias fold)."""
    row = np.concatenate([edges[0], np.arange(n, dtype=np.int64)])
    col = np.concatenate([edges[1], np.arange(n, dtype=np.int64)])
    deg = np.bincount(col, minlength=n).astype(np.float32)
    dinv = np.zeros(n, np.float32)
    nz = deg > 0
    dinv[nz] = 1.0 / np.sqrt(deg[nz])
    norm = dinv[row] * dinv[col]
    at = np.zeros((npad, npad), np.float32)
    np.add.at(at, (row, col), norm)
    if bias_row:
        at[n, :n] = 1.0
    return at.astype(NP_BF16)


def _tile_rows(a, kt):
    """[kt*P, F] -> [P, kt, F] so that [p, k, :] = a[k*P + p, :]."""
    return np.ascontiguousarray(
        a.reshape(kt, P, a.shape[-1]).transpose(1, 0, 2))


_PROGRAM_CACHE = {}


def _get_program(bl, n, c):
    key = (bl, n, c)
    if key not in _PROGRAM_CACHE:
        _PROGRAM_CACHE[key] = build_program(bl, n, c)
    return _PROGRAM_CACHE[key]


def run(inputs, trace=False, n_cores=N_CORES):
    x = np.asarray(inputs["x"], dtype=np.float32).astype(NP_BF16)
    w1 = np.asarray(inputs["W1"], np.float32)
    w2 = np.asarray(inputs["W2"], np.float32)
    w3 = np.asarray(inputs["W3"], np.float32)
    b1 = np.asarray(inputs["b1"], np.float32)
    b2 = np.asarray(inputs["b2"], np.float32)
    b3 = np.asarray(inputs["b3"], np.float32)
    e_sp = np.asarray(inputs["keypoint_line_without_temporal"]).astype(np.int64)
    e_tm = np.asarray(inputs["keypoint_line_with_temporal"]).astype(np.int64)

    b_total, n, c = x.shape
    bl = b_total // n_cores
    kt = -(-(n + 1) // P)
    npad = kt * P
    ct = c // P

    nc = _get_program(bl, n, c)

    at_sp = _tile_rows(_norm_adj_T(e_sp, n, npad, bias_row=True)[:, :n], kt)
    at_tm = _tile_rows(_norm_adj_T(e_tm, n, npad, bias_row=False)[:, :n], kt)
    shared = {
        "at_sp": at_sp,
        "at_tm": at_tm,
        "w1": _tile_rows(w1.astype(NP_BF16), ct),
        "w2": _tile_rows(w2.astype(NP_BF16), ct),
        "w3": _tile_rows(w3.astype(NP_BF16), ct),
        "b1": np.ascontiguousarray(b1.reshape(ct, P).T),
        "b2": np.ascontiguousarray(b2.reshape(ct, P).T),
        "b3": np.ascontiguousarray(b3.astype(NP_BF16)[None, :]),
    }
    in_maps = [
        {"x": np.ascontiguousarray(x[i * bl:(i + 1) * bl]), **shared}
        for i in range(n_cores)
    ]
    res = run_bass_kernel_spmd(nc, in_maps, core_ids=list(range(n_cores)),
                               trace=trace)
    out = np.concatenate([r["out"] for r in res.results], axis=0)
    return out, res


def kernel(**inputs) -> np.ndarray:
    out, _ = run(inputs, trace=False)
    return out


# revision 28
# speedup vs baseline: 1.0569x; 1.0016x over previous
"""Trainium2 Bass kernel for nn_BasicBlock (3-layer GCN block with residual).

Math (per batch item b, per conv):
    out = A @ (x @ W) + bias,  A = normalized adjacency (with self loops)
where A[c, r] = sum over edges r->c of dinv[r]*dinv[c] (dense N x N, shared
across batch and precomputed on host from the edge lists).

Block:
    a1 = relu(A_sp @ (x  @ W1) + b1)
    a2 = relu(A_tm @ (a1 @ W2) + b2)
    o3 =      A_sp @ (a2 @ W3) + b3
    out = relu(o3 + x)

On-chip layouts per item (P=128 partitions):
    natural  [n, c] : node chunks on partitions           (rhs of A-matmul /
                                                           lhsT of form-iv)
    transposed [c, n]: channel chunks on partitions        (consumed by W-matmul)

Phases per item (matmul forms; AT = A^T so AT[m, n] = A[n, m]):
    1. g1T[c,n]  = sum_m x[m,c]  * AT_sp[m,n]      (lhsT=x chunk,  rhs=AT_sp)
    2. a1T[co,n] = relu(sum_ci W1[ci,co]*g1T[ci,n] + b1)   (lhsT=W1, rhs=g1T)
    3. h2[n,c]   = sum_ci a1T[ci,n] * W2[ci,c]     (lhsT=a1T chunk, rhs=W2)
    4. a2T[c,n]  = relu(sum_m h2[m,c]*AT_tm[m,n] + b2)
    5. h3[n,c]   = sum_ci a2T[ci,n] * W3[ci,c];  h3[N,:] = b3
    6. out[n,c]  = relu(sum_m AT_sp[m,n]*h3[m,c] + x[n,c])
       (AT_sp row N is all-ones over valid cols -> adds b3 to every node;
        harmless in phase 1 because x row N is zero-padded)

All matmuls bf16 (1 cycle/row on PE) with fp32 PSUM accumulation; x arrives
pre-cast to bf16 from the host and doubles as the phase-6 residual. Batch
(64) is sharded 8 items/core over the 8 cores; A/W/b are replicated.
"""

import sys

if "/opt/trn_rl_repo" not in sys.path:
    sys.path.insert(0, "/opt/trn_rl_repo")

import numpy as np
import ml_dtypes

import concourse.bass as bass
import concourse.bacc as bacc
import concourse.mybir as mybir
import concourse.tile as tile
from concourse.bass_utils import run_bass_kernel_spmd

P = 128
B, N, C = 64, 1700, 256
N_CORES = 8
B_LOCAL = B // N_CORES

F32 = mybir.dt.float32
BF16 = mybir.dt.bfloat16
RELU = mybir.ActivationFunctionType.Relu
NP_BF16 = ml_dtypes.bfloat16


def _quarters(total, step=512):
    return [(q, min(step, total - q)) for q in range(0, total, step)]


def build_program(bl, n, c):
    """Build the Bass/Tile program for `bl` batch items, `n` nodes, `c` chans."""
    kt = -(-(n + 1) // P)  # node chunks; >= one pad row (bias row at index n)
    npad = kt * P
    ct = c // P
    nq = _quarters(npad)

    nqv = _quarters(n)  # valid-column quarters (phases whose pads are unread)

    nc = bacc.Bacc("TRN2", target_bir_lowering=False, debug=False,
                   enable_asserts=False)

    x_d = nc.dram_tensor("x", [bl, n, c], BF16, kind="ExternalInput")
    atsp_d = nc.dram_tensor("at_sp", [P, kt, n], BF16, kind="ExternalInput")
    attm_d = nc.dram_tensor("at_tm", [P, kt, n], BF16, kind="ExternalInput")
    w_d = [nc.dram_tensor(f"w{i}", [P, ct, c], BF16, kind="ExternalInput")
           for i in (1, 2, 3)]
    b1_d = nc.dram_tensor("b1", [P, ct], F32, kind="ExternalInput")
    b2_d = nc.dram_tensor("b2", [P, ct], F32, kind="ExternalInput")
    b3_d = nc.dram_tensor("b3", [1, c], BF16, kind="ExternalInput")
    out_d = nc.dram_tensor("out", [bl, n, c], F32, kind="ExternalOutput")

    with tile.TileContext(nc) as tc:
        with (
            tc.tile_pool(name="const", bufs=1) as cpool,
            tc.tile_pool(name="xbf", bufs=4) as xbfp,
            tc.tile_pool(name="act", bufs=4) as actp,
            tc.tile_pool(name="h", bufs=3) as hp,
            tc.tile_pool(name="outp", bufs=4) as outp,
            tc.tile_pool(name="psA", bufs=4, space="PSUM") as psA,
            tc.tile_pool(name="psW", bufs=4, space="PSUM") as psW,
        ):
            # --- constants.  Ring plan: at_sp is needed first (item-0
            # phase 1 consumes tile k at ~1.4*k us), so every tile is split
            # across the sync+scalar HWDGE rings, with at_tm queued behind
            # it; x for items 0-1 rides the gpsimd SWDGE ring, later items
            # the sync ring; out stores go on scalar. ---
            at_sp = cpool.tile([P, kt, n], BF16, tag="at_sp")
            at_tm = cpool.tile([P, kt, n], BF16, tag="at_tm")
            nh = n // 2
            for k in range(kt):
                # split every tile across both HWDGE rings so tile k
                # completes at ~1.2*(k+1) us, tracking PE consumption
                nc.sync.dma_start(at_sp[:, k, :nh], atsp_d[:, k, :nh])
                nc.scalar.dma_start(at_sp[:, k, nh:], atsp_d[:, k, nh:])

            w_sb = []
            for i, wd in enumerate(w_d):
                w = cpool.tile([P, ct, c], BF16, tag=f"w{i}")
                nc.scalar.dma_start(w[:], wd[:])
                w_sb.append(w)
            b1_sb = cpool.tile([P, ct], F32, tag="b1")
            b2_sb = cpool.tile([P, ct], F32, tag="b2")
            nc.scalar.dma_start(b1_sb[:], b1_d[:])
            nc.scalar.dma_start(b2_sb[:], b2_d[:])

            def emit_load_at_tm():
                # queued on the rings behind at_sp (and behind item-1's x on
                # sync) -- needed only from item-0 phase 4 (~52us in)
                for k in range(kt):
                    nc.sync.dma_start(at_tm[:, k, :nh], attm_d[:, k, :nh])
                    nc.scalar.dma_start(at_tm[:, k, nh:], attm_d[:, k, nh:])

            bias_tile = n // P      # global node index n == first pad row
            bias_part = n % P

            def emit_load_x(b, eng=None):
                # x arrives pre-cast bf16 from the host; DMA straight into
                # the padded [P, kt, c] tile (pad rows zeroed)
                x_eng = eng if eng is not None else (
                    nc.gpsimd if b <= 1 else nc.sync)
                xbf = xbfp.tile([P, kt, c], BF16, tag="xbf", name=f"xbf_{b}")
                for k in range(kt):
                    rows = min(P, n - k * P)
                    if rows < P:
                        nc.vector.memset(xbf[:, k, :], 0)
                    if rows > 0:
                        x_eng.dma_start(xbf[:rows, k, :],
                                        x_d[b, k * P:k * P + rows, :])
                return xbf

            def emit_p1(b, xbf):
                # phase 1: g1T = (A_sp @ x)^T
                g1T = actp.tile([P, ct, npad], BF16, tag="act", name=f"g1T_{b}")
                if b == 0:
                    # k-outer over 8 parallel PSUM banks so tile k of at_sp
                    # is consumed as soon as its DMA lands
                    groups = []
                    for cc in range(ct):
                        for qi, (q0, qs) in enumerate(nqv):
                            pool, tg = ((psA, "psA")
                                        if (cc * len(nqv) + qi) % 2 == 0
                                        else (psW, "psW"))
                            groups.append(
                                (pool.tile([P, 512], F32, tag=tg,
                                           name=f"ps1_{cc}_{qi}"), cc, q0, qs))
                    for k in range(kt):
                        for (ps, cc, q0, qs) in groups:
                            nc.tensor.matmul(
                                ps[:, :qs],
                                lhsT=xbf[:, k, cc * P:(cc + 1) * P],
                                rhs=at_sp[:, k, q0:q0 + qs],
                                start=(k == 0), stop=(k == kt - 1))
                    for (ps, cc, q0, qs) in groups:
                        nc.vector.tensor_copy(g1T[:, cc, q0:q0 + qs], ps[:, :qs])
                else:
                    for cc in range(ct):
                        for (q0, qs) in nqv:
                            ps = psA.tile([P, 512], F32, tag="psA")
                            for k in range(kt):
                                nc.tensor.matmul(
                                    ps[:, :qs],
                                    lhsT=xbf[:, k, cc * P:(cc + 1) * P],
                                    rhs=at_sp[:, k, q0:q0 + qs],
                                    start=(k == 0), stop=(k == kt - 1))
                            nc.vector.tensor_copy(g1T[:, cc, q0:q0 + qs],
                                                  ps[:, :qs])
                return g1T

            def emit_p2(b, g1T):
                # phase 2: a1T = relu(W1^T @ g1T + b1)
                a1T = actp.tile([P, ct, npad], BF16, tag="act", name=f"a1T_{b}")
                for cc in range(ct):
                    # cols [n:npad] are read as phase-3 lhsT pads but never
                    # written by the trimmed quarters
                    nc.vector.memset(a1T[:, cc, n:npad], 0)
                for co in range(ct):
                    for (q0, qs) in nqv:
                        ps = psA.tile([P, 512], F32, tag="psA")
                        for ci in range(ct):
                            nc.tensor.matmul(
                                ps[:, :qs],
                                lhsT=w_sb[0][:, ci, co * P:(co + 1) * P],
                                rhs=g1T[:, ci, q0:q0 + qs],
                                start=(ci == 0), stop=(ci == ct - 1))
                        nc.scalar.activation(a1T[:, co, q0:q0 + qs], ps[:, :qs],
                                             RELU, bias=b1_sb[:, co:co + 1])
                return a1T

            def emit_p3(b, a1T):
                # phase 3: h2 = a1 @ W2 (natural layout)
                h2 = hp.tile([P, kt, c], BF16, tag="h", name=f"h2_{b}")
                for k in range(kt):
                    ps = psW.tile([P, c], F32, tag="psW")
                    for ci in range(ct):
                        nc.tensor.matmul(
                            ps[:],
                            lhsT=a1T[:, ci, k * P:(k + 1) * P],
                            rhs=w_sb[1][:, ci, :],
                            start=(ci == 0), stop=(ci == ct - 1))
                    nc.vector.tensor_copy(h2[:, k, :], ps[:])
                return h2

            def emit_p4(b, h2):
                # phase 4: a2T = relu((A_tm @ h2)^T + b2)
                a2T = actp.tile([P, ct, npad], BF16, tag="act", name=f"a2T_{b}")
                for cc in range(ct):
                    nc.vector.memset(a2T[:, cc, n:npad], 0)
                if b == 0:
                    # k-outer in two 4-bank rounds so at_tm tiles are
                    # consumed while their DMAs are still landing
                    for cc in range(ct):
                        groups = [(psA.tile([P, 512], F32, tag="psA",
                                            name=f"ps4_{cc}_{q0}"), q0, qs)
                                  for (q0, qs) in nqv]
                        for k in range(kt):
                            for (ps, q0, qs) in groups:
                                nc.tensor.matmul(
                                    ps[:, :qs],
                                    lhsT=h2[:, k, cc * P:(cc + 1) * P],
                                    rhs=at_tm[:, k, q0:q0 + qs],
                                    start=(k == 0), stop=(k == kt - 1))
                        for (ps, q0, qs) in groups:
                            nc.scalar.activation(a2T[:, cc, q0:q0 + qs],
                                                 ps[:, :qs], RELU,
                                                 bias=b2_sb[:, cc:cc + 1])
                else:
                    for cc in range(ct):
                        for (q0, qs) in nqv:
                            ps = psA.tile([P, 512], F32, tag="psA")
                            for k in range(kt):
                                nc.tensor.matmul(
                                    ps[:, :qs],
                                    lhsT=h2[:, k, cc * P:(cc + 1) * P],
                                    rhs=at_tm[:, k, q0:q0 + qs],
                                    start=(k == 0), stop=(k == kt - 1))
                            nc.scalar.activation(a2T[:, cc, q0:q0 + qs],
                                                 ps[:, :qs], RELU,
                                                 bias=b2_sb[:, cc:cc + 1])
                return a2T

            def emit_p5(b, a2T):
                # phase 5: h3 = a2 @ W3; h3[row n] = b3
                h3 = hp.tile([P, kt, c], BF16, tag="h", name=f"h3_{b}")
                for k in range(kt):
                    ps = psW.tile([P, c], F32, tag="psW")
                    for ci in range(ct):
                        nc.tensor.matmul(
                            ps[:],
                            lhsT=a2T[:, ci, k * P:(k + 1) * P],
                            rhs=w_sb[2][:, ci, :],
                            start=(ci == 0), stop=(ci == ct - 1))
                    nc.vector.tensor_copy(h3[:, k, :], ps[:])
                nc.scalar.dma_start(
                    h3[bias_part:bias_part + 1, bias_tile, :], b3_d[:, :])
                return h3

            def emit_p6(b, xbf, h3):
                # phase 6: out = relu(A_sp @ h3 + x), residual from the
                # resident bf16 x tile
                for ko in range(kt):
                    rows = min(P, n - ko * P)
                    if rows <= 0:
                        continue
                    ps = psW.tile([P, c], F32, tag="psW")
                    for k in range(kt):
                        nc.tensor.matmul(
                            ps[:rows, :],
                            lhsT=at_sp[:, k, ko * P:ko * P + rows],
                            rhs=h3[:, k, :],
                            start=(k == 0), stop=(k == kt - 1))
                    ot = outp.tile([P, c], F32, tag="o")
                    nc.vector.tensor_add(ot[:rows, :], ps[:rows, :],
                                         xbf[:rows, ko, :])
                    nc.scalar.activation(ot[:rows, :], ot[:rows, :], RELU)
                    nc.scalar.dma_start(out_d[b, ko * P:ko * P + rows, :],
                                        ot[:rows, :])

            def emit_item(b, xbf=None, g1T=None):
                if xbf is None:
                    xbf = emit_load_x(b)
                if g1T is None:
                    g1T = emit_p1(b, xbf)
                a1T = emit_p2(b, g1T)
                h2 = emit_p3(b, a1T)
                a2T = emit_p4(b, h2)
                h3 = emit_p5(b, a2T)
                emit_p6(b, xbf, h3)

            # Emission order: item-1 phase 1 is hoisted between item-0
            # phase 1 and phase 2 so the PE has ~21us more matmul work
            # before the first at_tm use (its DMA trails at_sp).
            xbf0 = emit_load_x(0)
            g1T0 = emit_p1(0, xbf0)
            if bl > 1:
                xbf1 = emit_load_x(1)
                emit_load_at_tm()
                g1T1 = emit_p1(1, xbf1)
            else:
                emit_load_at_tm()
            emit_item(0, xbf0, g1T0)
            if bl > 1:
                emit_item(1, xbf1, g1T1)
            for b in range(2, bl):
                emit_item(b)

    nc.compile()
    return nc


def _norm_adj_T(edges, n, npad, bias_row):
    """A^T padded to [npad, npad] in bf16. AT[m, j] = A[j, m] where
    out[j] += A[j, m] * h[m]; edge (r -> c) contributes dinv[r]*dinv[c] at
    AT[r, c]. Self loops included. If bias_row, AT[n, :n] = 1 (bias fold)."""
    row = np.concatenate([edges[0], np.arange(n, dtype=np.int64)])
    col = np.concatenate([edges[1], np.arange(n, dtype=np.int64)])
    deg = np.bincount(col, minlength=n).astype(np.float32)
    dinv = np.zeros(n, np.float32)
    nz = deg > 0
    dinv[nz] = 1.0 / np.sqrt(deg[nz])
    norm = dinv[row] * dinv[col]
    at = np.zeros((npad, npad), np.float32)
    np.add.at(at, (row, col), norm)
    if bias_row:
        at[n, :n] = 1.0
    return at.astype(NP_BF16)


def _tile_rows(a, kt):
    """[kt*P, F] -> [P, kt, F] so that [p, k, :] = a[k*P + p, :]."""
    return np.ascontiguousarray(
        a.reshape(kt, P, a.shape[-1]).transpose(1, 0, 2))


_PROGRAM_CACHE = {}


def _get_program(bl, n, c):
    key = (bl, n, c)
    if key not in _PROGRAM_CACHE:
        _PROGRAM_CACHE[key] = build_program(bl, n, c)
    return _PROGRAM_CACHE[key]


def run(inputs, trace=False, n_cores=N_CORES):
    x = np.asarray(inputs["x"], dtype=np.float32).astype(NP_BF16)
    w1 = np.asarray(inputs["W1"], np.float32)
    w2 = np.asarray(inputs["W2"], np.float32)
    w3 = np.asarray(inputs["W3"], np.float32)
    b1 = np.asarray(inputs["b1"], np.float32)
    b2 = np.asarray(inputs["b2"], np.float32)
    b3 = np.asarray(inputs["b3"], np.float32)
    e_sp = np.asarray(inputs["keypoint_line_without_temporal"]).astype(np.int64)
    e_tm = np.asarray(inputs["keypoint_line_with_temporal"]).astype(np.int64)

    b_total, n, c = x.shape
    bl = b_total // n_cores
    kt = -(-(n + 1) // P)
    npad = kt * P
    ct = c // P

    nc = _get_program(bl, n, c)

    at_sp = _tile_rows(_norm_adj_T(e_sp, n, npad, bias_row=True)[:, :n], kt)
    at_tm = _tile_rows(_norm_adj_T(e_tm, n, npad, bias_row=False)[:, :n], kt)
    shared = {
        "at_sp": at_sp,
        "at_tm": at_tm,
        "w1": _tile_rows(w1.astype(NP_BF16), ct),
        "w2": _tile_rows(w2.astype(NP_BF16), ct),
        "w3": _tile_rows(w3.astype(NP_BF16), ct),
        "b1": np.ascontiguousarray(b1.reshape(ct, P).T),
        "b2": np.ascontiguousarray(b2.reshape(ct, P).T),
        "b3": np.ascontiguousarray(b3.astype(NP_BF16)[None, :]),
    }
    in_maps = [
        {"x": np.ascontiguousarray(x[i * bl:(i + 1) * bl]), **shared}
        for i in range(n_cores)
    ]
    res = run_bass_kernel_spmd(nc, in_maps, core_ids=list(range(n_cores)),
                               trace=trace)
    out = np.concatenate([r["out"] for r in res.results], axis=0)
    return out, res


def kernel(**inputs) -> np.ndarray:
    out, _ = run(inputs, trace=False)
    return out
